# revision 6
# baseline (speedup 1.0000x reference)
"""GNN NodeModel kernel for 8 Trainium2 NeuronCores.

Strategy: shard edges by DESTINATION node block (512 nodes), so scatter_mean
is fully core-local (no collectives). Per core:
  - edge-parallel MLP1 (feature-major activations, float32r matmuls)
  - scatter-add via one-hot S-matrix matmuls into per-node-block PSUM
    accumulators (S built on device with iota + is_equal from dest indices)
  - scatter_mean division folded into the PSUM->SBUF eviction (x inv count)
  - node-parallel MLP2 on the aggregated features
All 8 cores run one shared SPMD program; per-node-block edge counts are made
structurally identical across cores by sorting blocks by edge count (LPT
assignment) and padding each rank to the max across cores.
"""

import os
import sys

sys.path.insert(0, "/opt/trn_rl_repo")

import numpy as np

import concourse.bass as bass
import concourse.mybir as mybir
import concourse.tile as tile
from concourse import bacc
from concourse.bass_utils import run_bass_kernel_spmd

P = 128          # partitions
H = 512          # hidden width
NBN = 512        # nodes per node-block (scatter + MLP2 unit)
EB = 512         # edges per compute block
NCORES = 8

F32 = mybir.dt.float32
F32R = mybir.dt.float32r
I32 = mybir.dt.int32

LAST_RUN_INFO = {}


def _build_structure(row, n_nodes):
    """Partition node blocks across cores; compute shared slot structure."""
    n_blocks_g = -(-n_nodes // NBN)
    bcnt = np.bincount(row // NBN, minlength=n_blocks_g).astype(np.int64)
    pcnt = np.maximum(P, ((bcnt + P - 1) // P) * P)

    # LPT assignment of global blocks to cores, balancing padded edge counts
    order_desc = np.argsort(-pcnt, kind="stable")
    core_blocks = [[] for _ in range(NCORES)]
    core_tot = np.zeros(NCORES, dtype=np.int64)
    for g in order_desc:
        k = int(np.argmin(core_tot))
        core_blocks[k].append(int(g))
        core_tot[k] += pcnt[g]

    nbk = max(len(bl) for bl in core_blocks)
    # per-rank slot capacity = max padded count across cores at that rank
    C = np.full(nbk, P, dtype=np.int64)
    for bl in core_blocks:
        for j, g in enumerate(bl):
            C[j] = max(C[j], pcnt[g])
    et = int(C.sum())
    rem = (-et) % EB
    C[-1] += rem
    et += rem
    return core_blocks, nbk, C, et, bcnt


def _build_program(nbk, C, et):
    """Trace the shared SPMD Bass program for the given slot structure."""
    sub = et // P
    ebk = et // EB
    npad = nbk * NBN

    # sub-tile t -> (block slot j, first?, last?)
    sub_first = {}
    sub_last = {}
    sub_blk = np.empty(sub, dtype=np.int64)
    t = 0
    for j in range(nbk):
        ns = int(C[j]) // P
        for s in range(ns):
            sub_blk[t] = j
            if s == 0:
                sub_first[t] = True
            if s == ns - 1:
                sub_last[t] = True
            t += 1
    assert t == sub

    nc = bacc.Bacc("TRN2", target_bir_lowering=False, debug=False)
    A0 = nc.declare_dram_parameter("a0", [P, 4, et], F32R, isOutput=False)
    A1 = nc.declare_dram_parameter("a1", [9, et], F32R, isOutput=False)
    DLOC = nc.declare_dram_parameter("dloc", [P, sub], F32, isOutput=False)
    INVB = nc.declare_dram_parameter("invb", [P, npad], F32, isOutput=False)
    XU = nc.declare_dram_parameter("xu", [25, npad], F32R, isOutput=False)
    W1AE = nc.declare_dram_parameter("w1ae", [P, 4, H], F32R, isOutput=False)
    W1AX = nc.declare_dram_parameter("w1ax", [9, H], F32R, isOutput=False)
    W1B = nc.declare_dram_parameter("w1b", [P, 4, H], F32R, isOutput=False)
    W2AA = nc.declare_dram_parameter("w2aa", [P, 4, H], F32R, isOutput=False)
    W2AX = nc.declare_dram_parameter("w2ax", [25, H], F32R, isOutput=False)
    W2B = nc.declare_dram_parameter("w2b", [P, 4], F32R, isOutput=False)
    B1A = nc.declare_dram_parameter("b1a", [P, 4], F32, isOutput=False)
    B2A = nc.declare_dram_parameter("b2a", [P, 4], F32, isOutput=False)
    B1B = nc.declare_dram_parameter("b1b", [P, H], F32, isOutput=False)
    OUT = nc.declare_dram_parameter("out", [1, npad], F32, isOutput=True)

    with tile.TileContext(nc) as tc:
        with (
            tc.tile_pool(name="wpool", bufs=1) as wpool,
            tc.tile_pool(name="apool", bufs=3) as apool,
            tc.tile_pool(name="hpool", bufs=2) as hpool,
            tc.tile_pool(name="h2pool", bufs=6) as h2pool,
            tc.tile_pool(name="spool", bufs=4) as spool,
            tc.tile_pool(name="ztpool", bufs=2) as ztpool,
            tc.tile_pool(name="ttpool", bufs=2) as ttpool,
            tc.tile_pool(name="mmps", bufs=3, space="PSUM") as mmps,
            tc.tile_pool(name="aggps", bufs=1, space="PSUM") as aggps,
            tc.tile_pool(name="outps", bufs=1, space="PSUM") as outps,
        ):
            # ---- constants / weights ----
            w1ae = wpool.tile([P, 4, H], F32R)
            nc.sync.dma_start(w1ae[:], W1AE[:])
            w1ax = wpool.tile([9, H], F32R)
            nc.sync.dma_start(w1ax[:], W1AX[:])
            w1b = wpool.tile([P, 4, H], F32R)
            nc.sync.dma_start(w1b[:], W1B[:])
            w2aa = wpool.tile([P, 4, H], F32R)
            nc.sync.dma_start(w2aa[:], W2AA[:])
            w2ax = wpool.tile([25, H], F32R)
            nc.sync.dma_start(w2ax[:], W2AX[:])
            w2b = wpool.tile([P, 4], F32R)
            nc.sync.dma_start(w2b[:], W2B[:])
            b1a = wpool.tile([P, 4], F32)
            nc.sync.dma_start(b1a[:], B1A[:])
            b2a = wpool.tile([P, 4], F32)
            nc.sync.dma_start(b2a[:], B2A[:])
            b1b = wpool.tile([P, H], F32)
            nc.sync.dma_start(b1b[:], B1B[:])
            dloc = wpool.tile([P, sub], F32)
            nc.sync.dma_start(dloc[:], DLOC[:])
            invb = wpool.tile([P, npad], F32)
            nc.sync.dma_start(invb[:], INVB[:])

            iota_i = wpool.tile([P, NBN], I32)
            nc.gpsimd.iota(iota_i[:], pattern=[[1, NBN]], base=0, channel_multiplier=0)
            iota_f = wpool.tile([P, NBN], F32)
            nc.vector.tensor_copy(iota_f[:], iota_i[:])

            out_row = wpool.tile([1, npad], F32)

            agg = [None] * 4  # live aggT psum tiles (one per h-chunk)

            def mlp2(j, zt):
                xu = apool.tile([25, NBN], F32R, name="xu")
                nc.sync.dma_start(xu[:], XU[:, j * NBN:(j + 1) * NBN])
                tts = []
                for m in range(4):
                    pst = mmps.tile([P, H], F32, tag="mm")
                    for k in range(4):
                        nc.tensor.matmul(
                            pst[:], w2aa[:, k, m * P:(m + 1) * P], zt[:, k, :],
                            start=(k == 0), stop=False,
                        )
                    nc.tensor.matmul(
                        pst[:], w2ax[:, m * P:(m + 1) * P], xu[:],
                        start=False, stop=True,
                    )
                    tt = ttpool.tile([P, NBN], F32R, name=f"tt{m}")
                    nc.scalar.activation(
                        tt[:], pst[:], mybir.ActivationFunctionType.Relu,
                        bias=b2a[:, m:m + 1],
                    )
                    tts.append(tt)
                ops = outps.tile([1, NBN], F32, tag="outps")
                for k in range(4):
                    nc.tensor.matmul(
                        ops[:], w2b[:, k:k + 1], tts[k][:],
                        start=(k == 0), stop=(k == 3),
                    )
                nc.vector.tensor_copy(out_row[0:1, j * NBN:(j + 1) * NBN], ops[:])

            # ---- main loop over edge blocks ----
            for b in range(ebk):
                a0 = apool.tile([P, 4, EB], F32R, name="a0")
                nc.sync.dma_start(a0[:], A0[:, :, b * EB:(b + 1) * EB])
                a1 = apool.tile([9, EB], F32R, name="a1")
                nc.sync.dma_start(a1[:], A1[:, b * EB:(b + 1) * EB])

                # MLP1 layer 1: h1T[m] = relu(W1a[:,m].T @ A + b1a[m]), h-major
                h1 = []
                for m in range(4):
                    ps = mmps.tile([P, EB], F32, tag="mm")
                    for k in range(4):
                        nc.tensor.matmul(
                            ps[:], w1ae[:, k, m * P:(m + 1) * P], a0[:, k, :],
                            start=(k == 0), stop=False,
                        )
                    nc.tensor.matmul(
                        ps[:], w1ax[:, m * P:(m + 1) * P], a1[:],
                        start=False, stop=True,
                    )
                    h1t = hpool.tile([P, EB], F32R, name=f"h1t{m}")
                    nc.scalar.activation(
                        h1t[:], ps[:], mybir.ActivationFunctionType.Relu,
                        bias=b1a[:, m:m + 1],
                    )
                    h1.append(h1t)

                # MLP1 layer 2: h2[es] = h1.T @ W1b + b1b, edge-major
                h2 = []
                for es in range(4):
                    ps2 = mmps.tile([P, H], F32, tag="mm")
                    for k in range(4):
                        nc.tensor.matmul(
                            ps2[:], h1[k][:, es * P:(es + 1) * P], w1b[:, k, :],
                            start=(k == 0), stop=(k == 3),
                        )
                    h2t = h2pool.tile([P, H], F32R, name="h2")
                    nc.vector.tensor_tensor(
                        out=h2t[:], in0=ps2[:], in1=b1b[:], op=mybir.AluOpType.add,
                    )
                    h2.append(h2t)

                # scatter: aggT[m] += h2[:, m-chunk].T @ S
                for es in range(4):
                    t = b * 4 + es
                    j = int(sub_blk[t])
                    s_t = spool.tile([P, NBN], F32R, name="s")
                    nc.vector.tensor_scalar(
                        out=s_t[:], in0=iota_f[:], scalar1=dloc[:, t:t + 1],
                        scalar2=None, op0=mybir.AluOpType.is_equal,
                    )
                    first = sub_first.get(t, False)
                    last = sub_last.get(t, False)
                    if first:
                        for m in range(4):
                            agg[m] = aggps.tile([P, NBN], F32, tag=f"agg{m}", name=f"agg{m}")
                    for m in range(4):
                        nc.tensor.matmul(
                            agg[m][:], h2[es][:, m * P:(m + 1) * P], s_t[:],
                            start=first, stop=last, skip_group_check=True,
                        )
                    if last:
                        # evict aggT -> zT with scatter_mean division folded in
                        zt = ztpool.tile([P, 4, NBN], F32R, name="zt")
                        for m in range(4):
                            nc.vector.tensor_tensor(
                                out=zt[:, m, :], in0=agg[m][:],
                                in1=invb[:, j * NBN:(j + 1) * NBN],
                                op=mybir.AluOpType.mult,
                            )
                        mlp2(j, zt)

            nc.sync.dma_start(OUT[:], out_row[:])

    nc.compile()
    return nc


def kernel(**inputs):
    x = np.ascontiguousarray(np.asarray(inputs["x"], dtype=np.float32))
    edge_index = np.asarray(inputs["edge_index"], dtype=np.int64)
    edge_attr = np.ascontiguousarray(np.asarray(inputs["edge_attr"], dtype=np.float32))
    u = np.asarray(inputs["u"], dtype=np.float32)
    batch = np.asarray(inputs["batch"], dtype=np.int64)
    W1a = np.asarray(inputs["W1a"], dtype=np.float32)
    b1a = np.asarray(inputs["b1a"], dtype=np.float32)
    W1b = np.asarray(inputs["W1b"], dtype=np.float32)
    b1b = np.asarray(inputs["b1b"], dtype=np.float32)
    W2a = np.asarray(inputs["W2a"], dtype=np.float32)
    b2a = np.asarray(inputs["b2a"], dtype=np.float32)
    W2b = np.asarray(inputs["W2b"], dtype=np.float32)
    b2b = np.asarray(inputs["b2b"], dtype=np.float32)

    n_nodes = x.shape[0]
    n_edges = edge_index.shape[1]
    row, col = edge_index[0], edge_index[1]

    cnt = np.bincount(row, minlength=n_nodes)
    inv = (1.0 / np.maximum(cnt, 1)).astype(np.float32)

    core_blocks, nbk, C, et, bcnt = _build_structure(row, n_nodes)
    sub = et // P
    npad = nbk * NBN
    Cstart = np.concatenate([[0], np.cumsum(C)])

    nc = _build_program(nbk, C, et)

    # ---- per-core shards ----
    order = np.argsort(row, kind="stable")
    bstart = np.concatenate([[0], np.cumsum(bcnt)])

    # weights (shared by all cores)
    W1a_e = np.ascontiguousarray(W1a[9:521].reshape(4, P, H).transpose(1, 0, 2))
    W1a_x = np.ascontiguousarray(W1a[0:9])
    W1b_r = np.ascontiguousarray(W1b.reshape(4, P, H).transpose(1, 0, 2))
    W2a_a = np.ascontiguousarray(W2a[9:521].reshape(4, P, H).transpose(1, 0, 2))
    W2a_x = np.ascontiguousarray(np.vstack([W2a[0:9], W2a[521:537]]))
    W2b_r = np.ascontiguousarray(W2b[:, 0].reshape(4, P).T)
    b1a_r = np.ascontiguousarray(b1a.reshape(4, P).T)
    b2a_r = np.ascontiguousarray(b2a.reshape(4, P).T)
    b1b_f = np.ascontiguousarray(np.tile(b1b[None, :], (P, 1)))

    xT = x.T  # [9, N]
    uT_b = u[batch].T  # [16, N]
    ea_T = edge_attr  # gathered below

    in_maps = []
    core_slot_blocks = []
    for k in range(NCORES):
        blocks = core_blocks[k] + [-1] * (nbk - len(core_blocks[k]))
        core_slot_blocks.append(blocks)
        eidx = np.full(et, -1, dtype=np.int64)
        for j, g in enumerate(blocks):
            if g >= 0:
                nege = int(bcnt[g])
                eidx[Cstart[j]:Cstart[j] + nege] = order[bstart[g]:bstart[g] + nege]
        valid = eidx >= 0
        e_safe = np.where(valid, eidx, 0)

        ea = edge_attr[e_safe]  # [et, 512]
        A0 = np.ascontiguousarray(ea.T.reshape(4, P, et).transpose(1, 0, 2))
        A1 = np.ascontiguousarray(x[col[e_safe]].T)  # [9, et]

        dl = np.full(et, -1.0, dtype=np.float32)
        dest = row[e_safe]
        # dest-local index within the slot's node block
        blk_of_slot = np.repeat(np.arange(nbk), C)
        gblk = np.array([blocks[j] for j in blk_of_slot], dtype=np.int64)
        dl_val = (dest - gblk * NBN).astype(np.float32)
        dl = np.where(valid, dl_val, -1.0).astype(np.float32)
        dloc_a = np.ascontiguousarray(dl.reshape(sub, P).T)  # [128, sub]

        invb_row = np.zeros(npad, dtype=np.float32)
        xu_a = np.zeros((25, npad), dtype=np.float32)
        for j, g in enumerate(blocks):
            if g < 0:
                continue
            lo = g * NBN
            hi = min(lo + NBN, n_nodes)
            w = hi - lo
            invb_row[j * NBN:j * NBN + w] = inv[lo:hi]
            xu_a[0:9, j * NBN:j * NBN + w] = xT[:, lo:hi]
            xu_a[9:25, j * NBN:j * NBN + w] = uT_b[:, lo:hi]
        invb_a = np.ascontiguousarray(np.broadcast_to(invb_row[None, :], (P, npad)))

        in_maps.append({
            "a0": A0, "a1": A1, "dloc": dloc_a, "invb": invb_a,
            "xu": np.ascontiguousarray(xu_a),
            "w1ae": W1a_e, "w1ax": W1a_x, "w1b": W1b_r,
            "w2aa": W2a_a, "w2ax": W2a_x, "w2b": W2b_r,
            "b1a": b1a_r, "b2a": b2a_r, "b1b": b1b_f,
        })

    res = run_bass_kernel_spmd(nc, in_maps, core_ids=list(range(NCORES)), trace=False)
    LAST_RUN_INFO.clear()
    LAST_RUN_INFO.update({
        "exec_time_ns": res.exec_time_ns,
        "nc": nc,
        "in_maps": in_maps,
    })

    out_full = np.zeros(n_nodes, dtype=np.float32)
    for k in range(NCORES):
        o = res.results[k]["out"][0]
        for j, g in enumerate(core_slot_blocks[k]):
            if g < 0:
                continue
            lo = g * NBN
            hi = min(lo + NBN, n_nodes)
            out_full[lo:hi] = o[j * NBN:j * NBN + (hi - lo)]

    result = out_full[:, None] + b2b[None, :] if b2b.ndim == 1 else out_full[:, None] + b2b
    return result.astype(np.float32)


def _bench_build(nc, in_maps, reps):
    """Build a jitted SPMD executable running the NEFF `reps` times back-to-back."""
    import jax
    import jax.numpy as jnp
    from jax.sharding import Mesh, PartitionSpec
    from jax.experimental.shard_map import shard_map

    from concourse import bass2jax
    from concourse import mybir as _mybir

    bass2jax.install_neuronx_cc_hook()
    partition_name = nc.partition_id_tensor.name if nc.partition_id_tensor else None

    in_names, out_names, out_avals, zero_outs = [], [], [], []
    for alloc in nc.m.functions[0].allocations:
        if not isinstance(alloc, _mybir.MemoryLocationSet):
            continue
        name = alloc.memorylocations[0].name
        if alloc.kind == "ExternalInput":
            if name != partition_name:
                in_names.append(name)
        elif alloc.kind == "ExternalOutput":
            shape = tuple(alloc.tensor_shape)
            dtype = _mybir.dt.np(alloc.dtype)
            out_names.append(name)
            out_avals.append(jax.core.ShapedArray(shape, dtype))
            zero_outs.append(np.zeros(shape, dtype))
    n_params = len(in_names)
    chain_idx = in_names.index("dloc")
    all_in_names = in_names + out_names
    if partition_name is not None:
        all_in_names.append(partition_name)

    bind_kw = dict(
        out_avals=tuple(out_avals),
        in_names=tuple(all_in_names),
        out_names=tuple(out_names),
        lowering_input_output_aliases=(),
        sim_require_finite=True,
        sim_require_nnan=True,
        nc=nc,
    )

    assert reps == 1

    def _body(*args):
        operands = list(args)
        if partition_name is not None:
            operands.append(bass2jax.partition_id_tensor())
        outs = bass2jax._bass_exec_p.bind(*operands, **bind_kw)
        return tuple(outs)

    n_cores = len(in_maps)
    devices = jax.devices()[:n_cores]
    mesh = Mesh(np.asarray(devices), ("core",))
    in_specs = (PartitionSpec("core"),) * (n_params + len(out_names))
    out_specs = (PartitionSpec("core"),) * len(out_names)
    fn = jax.jit(
        shard_map(_body, mesh=mesh, in_specs=in_specs, out_specs=out_specs,
                  check_rep=False),
        keep_unused=True,
    )
    concat_in = [
        np.concatenate([np.asarray(in_maps[c][nm]) for c in range(n_cores)], axis=0)
        for nm in in_names
    ] + [np.concatenate([z] * n_cores, axis=0) for z in zero_outs]
    args = [jax.device_put(a) for a in concat_in]
    return fn, args


def bench(n_pipe=32, iters=3):
    """Measure per-NEFF-execution wall time with pipelined async dispatches."""
    import time

    nc = LAST_RUN_INFO["nc"]
    in_maps = LAST_RUN_INFO["in_maps"]

    fn, args = _bench_build(nc, in_maps, 1)
    fn(*args)[0].block_until_ready()  # warm
    singles = []
    for _ in range(iters):
        t0 = time.perf_counter()
        fn(*args)[0].block_until_ready()
        singles.append(time.perf_counter() - t0)
    pipes = []
    for _ in range(iters):
        t0 = time.perf_counter()
        outs = [fn(*args) for _ in range(n_pipe)]
        outs[-1][0].block_until_ready()
        pipes.append((time.perf_counter() - t0) / n_pipe)
    exec_ns = min(pipes) * 1e9
    LAST_RUN_INFO["exec_time_ns"] = exec_ns
    LAST_RUN_INFO["bench_detail"] = {
        "single_ms": [f"{s * 1e3:.2f}" for s in singles],
        "piped_ms": [f"{p * 1e3:.2f}" for p in pipes],
    }
    return exec_ns


# revision 7
# speedup vs baseline: 10.7171x; 10.7171x over previous
"""GNN NodeModel kernel for 8 Trainium2 NeuronCores.

Strategy: shard edges by DESTINATION node block (512 nodes), so scatter_mean
is fully core-local (no collectives). Per core:
  - edge-parallel MLP1 (feature-major activations, float32r matmuls)
  - scatter-add via one-hot S-matrix matmuls into per-node-block PSUM
    accumulators (S built on device with iota + is_equal from dest indices)
  - scatter_mean division folded into the PSUM->SBUF eviction (x inv count)
  - node-parallel MLP2 on the aggregated features
All 8 cores run one shared SPMD program; per-node-block edge counts are made
structurally identical across cores by sorting blocks by edge count (LPT
assignment) and padding each rank to the max across cores.
"""

import os
import sys

sys.path.insert(0, "/opt/trn_rl_repo")

import numpy as np

import concourse.bass as bass
import concourse.mybir as mybir
import concourse.tile as tile
from concourse import bacc
from concourse.bass_utils import run_bass_kernel_spmd

P = 128          # partitions
H = 512          # hidden width
NBN = 512        # nodes per node-block (scatter + MLP2 unit)
EB = 512         # edges per compute block
NCORES = 8

F32 = mybir.dt.float32
F32R = mybir.dt.float32r
I32 = mybir.dt.int32

LAST_RUN_INFO = {}


def _build_structure(row, n_nodes):
    """Partition node blocks across cores; compute shared slot structure."""
    n_blocks_g = -(-n_nodes // NBN)
    bcnt = np.bincount(row // NBN, minlength=n_blocks_g).astype(np.int64)
    pcnt = np.maximum(P, ((bcnt + P - 1) // P) * P)

    # LPT assignment of global blocks to cores, balancing padded edge counts
    order_desc = np.argsort(-pcnt, kind="stable")
    core_blocks = [[] for _ in range(NCORES)]
    core_tot = np.zeros(NCORES, dtype=np.int64)
    for g in order_desc:
        k = int(np.argmin(core_tot))
        core_blocks[k].append(int(g))
        core_tot[k] += pcnt[g]

    nbk = max(len(bl) for bl in core_blocks)
    # per-rank slot capacity = max padded count across cores at that rank
    C = np.full(nbk, P, dtype=np.int64)
    for bl in core_blocks:
        for j, g in enumerate(bl):
            C[j] = max(C[j], pcnt[g])
    et = int(C.sum())
    rem = (-et) % EB
    C[-1] += rem
    et += rem
    return core_blocks, nbk, C, et, bcnt


def _build_program(nbk, C, et):
    """Trace the shared SPMD Bass program for the given slot structure."""
    sub = et // P
    ebk = et // EB
    npad = nbk * NBN

    # sub-tile t -> (block slot j, first?, last?)
    sub_first = {}
    sub_last = {}
    sub_blk = np.empty(sub, dtype=np.int64)
    t = 0
    for j in range(nbk):
        ns = int(C[j]) // P
        for s in range(ns):
            sub_blk[t] = j
            if s == 0:
                sub_first[t] = True
            if s == ns - 1:
                sub_last[t] = True
            t += 1
    assert t == sub

    nc = bacc.Bacc("TRN2", target_bir_lowering=False, debug=False)
    A0 = nc.declare_dram_parameter("a0", [P, 4, et], F32R, isOutput=False)
    A1 = nc.declare_dram_parameter("a1", [9, et], F32R, isOutput=False)
    DLOC = nc.declare_dram_parameter("dloc", [P, sub], F32, isOutput=False)
    INVB = nc.declare_dram_parameter("invb", [P, npad], F32, isOutput=False)
    XU = nc.declare_dram_parameter("xu", [25, npad], F32R, isOutput=False)
    W1AE = nc.declare_dram_parameter("w1ae", [P, 4, H], F32R, isOutput=False)
    W1AX = nc.declare_dram_parameter("w1ax", [9, H], F32R, isOutput=False)
    W1B = nc.declare_dram_parameter("w1b", [P, 4, H], F32R, isOutput=False)
    W2AA = nc.declare_dram_parameter("w2aa", [P, 4, H], F32R, isOutput=False)
    W2AX = nc.declare_dram_parameter("w2ax", [25, H], F32R, isOutput=False)
    W2B = nc.declare_dram_parameter("w2b", [P, 4], F32R, isOutput=False)
    B1A = nc.declare_dram_parameter("b1a", [P, 4], F32, isOutput=False)
    B2A = nc.declare_dram_parameter("b2a", [P, 4], F32, isOutput=False)
    B1B = nc.declare_dram_parameter("b1b", [P, H], F32, isOutput=False)
    OUT = nc.declare_dram_parameter("out", [1, npad], F32, isOutput=True)

    with tile.TileContext(nc) as tc:
        with (
            tc.tile_pool(name="wpool", bufs=1) as wpool,
            tc.tile_pool(name="apool", bufs=3) as apool,
            tc.tile_pool(name="hpool", bufs=2) as hpool,
            tc.tile_pool(name="h2pool", bufs=6) as h2pool,
            tc.tile_pool(name="spool", bufs=4) as spool,
            tc.tile_pool(name="ztpool", bufs=2) as ztpool,
            tc.tile_pool(name="ttpool", bufs=2) as ttpool,
            tc.tile_pool(name="mmps", bufs=3, space="PSUM") as mmps,
            tc.tile_pool(name="aggps", bufs=1, space="PSUM") as aggps,
            tc.tile_pool(name="outps", bufs=1, space="PSUM") as outps,
        ):
            # ---- constants / weights ----
            w1ae = wpool.tile([P, 4, H], F32R)
            nc.sync.dma_start(w1ae[:], W1AE[:])
            w1ax = wpool.tile([9, H], F32R)
            nc.sync.dma_start(w1ax[:], W1AX[:])
            w1b = wpool.tile([P, 4, H], F32R)
            nc.sync.dma_start(w1b[:], W1B[:])
            w2aa = wpool.tile([P, 4, H], F32R)
            nc.sync.dma_start(w2aa[:], W2AA[:])
            w2ax = wpool.tile([25, H], F32R)
            nc.sync.dma_start(w2ax[:], W2AX[:])
            w2b = wpool.tile([P, 4], F32R)
            nc.sync.dma_start(w2b[:], W2B[:])
            b1a = wpool.tile([P, 4], F32)
            nc.sync.dma_start(b1a[:], B1A[:])
            b2a = wpool.tile([P, 4], F32)
            nc.sync.dma_start(b2a[:], B2A[:])
            b1b = wpool.tile([P, H], F32)
            nc.sync.dma_start(b1b[:], B1B[:])
            dloc = wpool.tile([P, sub], F32)
            nc.sync.dma_start(dloc[:], DLOC[:])
            invb = wpool.tile([P, npad], F32)
            nc.sync.dma_start(invb[:], INVB[:])

            iota_i = wpool.tile([P, NBN], I32)
            nc.gpsimd.iota(iota_i[:], pattern=[[1, NBN]], base=0, channel_multiplier=0)
            iota_f = wpool.tile([P, NBN], F32)
            nc.vector.tensor_copy(iota_f[:], iota_i[:])

            out_row = wpool.tile([1, npad], F32)

            agg = [None] * 4  # live aggT psum tiles (one per h-chunk)

            def mlp2(j, zt):
                xu = apool.tile([25, NBN], F32R, name="xu")
                nc.sync.dma_start(xu[:], XU[:, j * NBN:(j + 1) * NBN])
                tts = []
                for m in range(4):
                    pst = mmps.tile([P, H], F32, tag="mm")
                    for k in range(4):
                        nc.tensor.matmul(
                            pst[:], w2aa[:, k, m * P:(m + 1) * P], zt[:, k, :],
                            start=(k == 0), stop=False,
                        )
                    nc.tensor.matmul(
                        pst[:], w2ax[:, m * P:(m + 1) * P], xu[:],
                        start=False, stop=True,
                    )
                    tt = ttpool.tile([P, NBN], F32R, name=f"tt{m}")
                    nc.scalar.activation(
                        tt[:], pst[:], mybir.ActivationFunctionType.Relu,
                        bias=b2a[:, m:m + 1],
                    )
                    tts.append(tt)
                ops = outps.tile([1, NBN], F32, tag="outps")
                for k in range(4):
                    nc.tensor.matmul(
                        ops[:], w2b[:, k:k + 1], tts[k][:],
                        start=(k == 0), stop=(k == 3),
                    )
                nc.vector.tensor_copy(out_row[0:1, j * NBN:(j + 1) * NBN], ops[:])

            # ---- main loop over edge blocks ----
            for b in range(ebk):
                a0 = apool.tile([P, 4, EB], F32R, name="a0")
                nc.sync.dma_start(a0[:], A0[:, :, b * EB:(b + 1) * EB])
                a1 = apool.tile([9, EB], F32R, name="a1")
                nc.sync.dma_start(a1[:], A1[:, b * EB:(b + 1) * EB])

                # MLP1 layer 1: h1T[m] = relu(W1a[:,m].T @ A + b1a[m]), h-major
                h1 = []
                for m in range(4):
                    ps = mmps.tile([P, EB], F32, tag="mm")
                    for k in range(4):
                        nc.tensor.matmul(
                            ps[:], w1ae[:, k, m * P:(m + 1) * P], a0[:, k, :],
                            start=(k == 0), stop=False,
                        )
                    nc.tensor.matmul(
                        ps[:], w1ax[:, m * P:(m + 1) * P], a1[:],
                        start=False, stop=True,
                    )
                    h1t = hpool.tile([P, EB], F32R, name=f"h1t{m}")
                    nc.scalar.activation(
                        h1t[:], ps[:], mybir.ActivationFunctionType.Relu,
                        bias=b1a[:, m:m + 1],
                    )
                    h1.append(h1t)

                # MLP1 layer 2: h2[es] = h1.T @ W1b + b1b, edge-major
                h2 = []
                for es in range(4):
                    ps2 = mmps.tile([P, H], F32, tag="mm")
                    for k in range(4):
                        nc.tensor.matmul(
                            ps2[:], h1[k][:, es * P:(es + 1) * P], w1b[:, k, :],
                            start=(k == 0), stop=(k == 3),
                        )
                    h2t = h2pool.tile([P, H], F32R, name="h2")
                    nc.vector.tensor_tensor(
                        out=h2t[:], in0=ps2[:], in1=b1b[:], op=mybir.AluOpType.add,
                    )
                    h2.append(h2t)

                # scatter: aggT[m] += h2[:, m-chunk].T @ S
                for es in range(4):
                    t = b * 4 + es
                    j = int(sub_blk[t])
                    s_t = spool.tile([P, NBN], F32R, name="s")
                    nc.vector.tensor_scalar(
                        out=s_t[:], in0=iota_f[:], scalar1=dloc[:, t:t + 1],
                        scalar2=None, op0=mybir.AluOpType.is_equal,
                    )
                    first = sub_first.get(t, False)
                    last = sub_last.get(t, False)
                    if first:
                        for m in range(4):
                            agg[m] = aggps.tile([P, NBN], F32, tag=f"agg{m}", name=f"agg{m}")
                    for m in range(4):
                        nc.tensor.matmul(
                            agg[m][:], h2[es][:, m * P:(m + 1) * P], s_t[:],
                            start=first, stop=last, skip_group_check=True,
                        )
                    if last:
                        # evict aggT -> zT with scatter_mean division folded in
                        zt = ztpool.tile([P, 4, NBN], F32R, name="zt")
                        for m in range(4):
                            nc.vector.tensor_tensor(
                                out=zt[:, m, :], in0=agg[m][:],
                                in1=invb[:, j * NBN:(j + 1) * NBN],
                                op=mybir.AluOpType.mult,
                            )
                        mlp2(j, zt)

            nc.sync.dma_start(OUT[:], out_row[:])

    nc.compile()
    return nc


def kernel(**inputs):
    x = np.ascontiguousarray(np.asarray(inputs["x"], dtype=np.float32))
    edge_index = np.asarray(inputs["edge_index"], dtype=np.int64)
    edge_attr = np.ascontiguousarray(np.asarray(inputs["edge_attr"], dtype=np.float32))
    u = np.asarray(inputs["u"], dtype=np.float32)
    batch = np.asarray(inputs["batch"], dtype=np.int64)
    W1a = np.asarray(inputs["W1a"], dtype=np.float32)
    b1a = np.asarray(inputs["b1a"], dtype=np.float32)
    W1b = np.asarray(inputs["W1b"], dtype=np.float32)
    b1b = np.asarray(inputs["b1b"], dtype=np.float32)
    W2a = np.asarray(inputs["W2a"], dtype=np.float32)
    b2a = np.asarray(inputs["b2a"], dtype=np.float32)
    W2b = np.asarray(inputs["W2b"], dtype=np.float32)
    b2b = np.asarray(inputs["b2b"], dtype=np.float32)

    n_nodes = x.shape[0]
    n_edges = edge_index.shape[1]
    row, col = edge_index[0], edge_index[1]

    cnt = np.bincount(row, minlength=n_nodes)
    inv = (1.0 / np.maximum(cnt, 1)).astype(np.float32)

    core_blocks, nbk, C, et, bcnt = _build_structure(row, n_nodes)
    sub = et // P
    npad = nbk * NBN
    Cstart = np.concatenate([[0], np.cumsum(C)])

    nc = _build_program(nbk, C, et)

    # ---- per-core shards ----
    order = np.argsort(row, kind="stable")
    bstart = np.concatenate([[0], np.cumsum(bcnt)])

    # weights (shared by all cores)
    W1a_e = np.ascontiguousarray(W1a[9:521].reshape(4, P, H).transpose(1, 0, 2))
    W1a_x = np.ascontiguousarray(W1a[0:9])
    W1b_r = np.ascontiguousarray(W1b.reshape(4, P, H).transpose(1, 0, 2))
    W2a_a = np.ascontiguousarray(W2a[9:521].reshape(4, P, H).transpose(1, 0, 2))
    W2a_x = np.ascontiguousarray(np.vstack([W2a[0:9], W2a[521:537]]))
    W2b_r = np.ascontiguousarray(W2b[:, 0].reshape(4, P).T)
    b1a_r = np.ascontiguousarray(b1a.reshape(4, P).T)
    b2a_r = np.ascontiguousarray(b2a.reshape(4, P).T)
    b1b_f = np.ascontiguousarray(np.tile(b1b[None, :], (P, 1)))

    xT = x.T  # [9, N]
    uT_b = u[batch].T  # [16, N]
    ea_T = edge_attr  # gathered below

    in_maps = []
    core_slot_blocks = []
    for k in range(NCORES):
        blocks = core_blocks[k] + [-1] * (nbk - len(core_blocks[k]))
        core_slot_blocks.append(blocks)
        eidx = np.full(et, -1, dtype=np.int64)
        for j, g in enumerate(blocks):
            if g >= 0:
                nege = int(bcnt[g])
                eidx[Cstart[j]:Cstart[j] + nege] = order[bstart[g]:bstart[g] + nege]
        valid = eidx >= 0
        e_safe = np.where(valid, eidx, 0)

        ea = edge_attr[e_safe]  # [et, 512]
        A0 = np.ascontiguousarray(ea.T.reshape(4, P, et).transpose(1, 0, 2))
        A1 = np.ascontiguousarray(x[col[e_safe]].T)  # [9, et]

        dl = np.full(et, -1.0, dtype=np.float32)
        dest = row[e_safe]
        # dest-local index within the slot's node block
        blk_of_slot = np.repeat(np.arange(nbk), C)
        gblk = np.array([blocks[j] for j in blk_of_slot], dtype=np.int64)
        dl_val = (dest - gblk * NBN).astype(np.float32)
        dl = np.where(valid, dl_val, -1.0).astype(np.float32)
        dloc_a = np.ascontiguousarray(dl.reshape(sub, P).T)  # [128, sub]

        invb_row = np.zeros(npad, dtype=np.float32)
        xu_a = np.zeros((25, npad), dtype=np.float32)
        for j, g in enumerate(blocks):
            if g < 0:
                continue
            lo = g * NBN
            hi = min(lo + NBN, n_nodes)
            w = hi - lo
            invb_row[j * NBN:j * NBN + w] = inv[lo:hi]
            xu_a[0:9, j * NBN:j * NBN + w] = xT[:, lo:hi]
            xu_a[9:25, j * NBN:j * NBN + w] = uT_b[:, lo:hi]
        invb_a = np.ascontiguousarray(np.broadcast_to(invb_row[None, :], (P, npad)))

        in_maps.append({
            "a0": A0, "a1": A1, "dloc": dloc_a, "invb": invb_a,
            "xu": np.ascontiguousarray(xu_a),
            "w1ae": W1a_e, "w1ax": W1a_x, "w1b": W1b_r,
            "w2aa": W2a_a, "w2ax": W2a_x, "w2b": W2b_r,
            "b1a": b1a_r, "b2a": b2a_r, "b1b": b1b_f,
        })

    res = run_bass_kernel_spmd(nc, in_maps, core_ids=list(range(NCORES)), trace=False)
    LAST_RUN_INFO.clear()
    LAST_RUN_INFO.update({
        "exec_time_ns": res.exec_time_ns,
        "nc": nc,
        "in_maps": in_maps,
    })

    out_full = np.zeros(n_nodes, dtype=np.float32)
    for k in range(NCORES):
        o = res.results[k]["out"][0]
        for j, g in enumerate(core_slot_blocks[k]):
            if g < 0:
                continue
            lo = g * NBN
            hi = min(lo + NBN, n_nodes)
            out_full[lo:hi] = o[j * NBN:j * NBN + (hi - lo)]

    result = out_full[:, None] + b2b[None, :] if b2b.ndim == 1 else out_full[:, None] + b2b
    return result.astype(np.float32)


def _bench_build(nc, in_maps, reps):
    """Build a jitted SPMD executable running the NEFF `reps` times back-to-back."""
    import jax
    import jax.numpy as jnp
    from jax.sharding import Mesh, PartitionSpec
    from jax.experimental.shard_map import shard_map

    from concourse import bass2jax
    from concourse import mybir as _mybir

    bass2jax.install_neuronx_cc_hook()
    partition_name = nc.partition_id_tensor.name if nc.partition_id_tensor else None

    in_names, out_names, out_avals, zero_outs = [], [], [], []
    for alloc in nc.m.functions[0].allocations:
        if not isinstance(alloc, _mybir.MemoryLocationSet):
            continue
        name = alloc.memorylocations[0].name
        if alloc.kind == "ExternalInput":
            if name != partition_name:
                in_names.append(name)
        elif alloc.kind == "ExternalOutput":
            shape = tuple(alloc.tensor_shape)
            dtype = _mybir.dt.np(alloc.dtype)
            out_names.append(name)
            out_avals.append(jax.core.ShapedArray(shape, dtype))
            zero_outs.append(np.zeros(shape, dtype))
    n_params = len(in_names)
    chain_idx = in_names.index("dloc")
    all_in_names = in_names + out_names
    if partition_name is not None:
        all_in_names.append(partition_name)

    bind_kw = dict(
        out_avals=tuple(out_avals),
        in_names=tuple(all_in_names),
        out_names=tuple(out_names),
        lowering_input_output_aliases=(),
        sim_require_finite=True,
        sim_require_nnan=True,
        nc=nc,
    )

    assert reps == 1

    def _body(*args):
        operands = list(args)
        if partition_name is not None:
            operands.append(bass2jax.partition_id_tensor())
        outs = bass2jax._bass_exec_p.bind(*operands, **bind_kw)
        return tuple(outs)

    n_cores = len(in_maps)
    devices = jax.devices()[:n_cores]
    mesh = Mesh(np.asarray(devices), ("core",))
    in_specs = (PartitionSpec("core"),) * (n_params + len(out_names))
    out_specs = (PartitionSpec("core"),) * len(out_names)
    fn = jax.jit(
        shard_map(_body, mesh=mesh, in_specs=in_specs, out_specs=out_specs,
                  check_rep=False),
        keep_unused=True,
    )
    concat_in = [
        np.concatenate([np.asarray(in_maps[c][nm]) for c in range(n_cores)], axis=0)
        for nm in in_names
    ] + [np.concatenate([z] * n_cores, axis=0) for z in zero_outs]
    sharding = jax.sharding.NamedSharding(mesh, PartitionSpec("core"))
    args = [jax.device_put(a, sharding) for a in concat_in]
    return fn, args


def bench(n_pipe=32, iters=3):
    """Measure per-NEFF-execution wall time with pipelined async dispatches."""
    import time

    nc = LAST_RUN_INFO["nc"]
    in_maps = LAST_RUN_INFO["in_maps"]

    fn, args = _bench_build(nc, in_maps, 1)
    fn(*args)[0].block_until_ready()  # warm
    singles = []
    for _ in range(iters):
        t0 = time.perf_counter()
        fn(*args)[0].block_until_ready()
        singles.append(time.perf_counter() - t0)
    pipes = []
    for _ in range(iters):
        t0 = time.perf_counter()
        outs = [fn(*args) for _ in range(n_pipe)]
        outs[-1][0].block_until_ready()
        pipes.append((time.perf_counter() - t0) / n_pipe)
    exec_ns = min(pipes) * 1e9
    LAST_RUN_INFO["exec_time_ns"] = exec_ns
    LAST_RUN_INFO["bench_detail"] = {
        "single_ms": [f"{s * 1e3:.2f}" for s in singles],
        "piped_ms": [f"{p * 1e3:.2f}" for p in pipes],
    }
    return exec_ns
